# revision 11
# baseline (speedup 1.0000x reference)
"""CRF loss (forward-algorithm partition function minus gold score) on 8
Trainium2 NeuronCores.

Strategy (data-parallel over batch, 32 sequences per core):
 - The CRF forward DP is run in the exp domain so each step is one
   weight-stationary PE matmul (alpha @ exp(transitions)) plus one DVE
   elementwise multiply by exp(feats_t).  A uniform per-step downshift c is
   folded into the bulk exp (ACT engine) to keep magnitudes bounded;
   periodic true normalization (colsum via ones-matmul, reciprocal,
   rank-1 broadcast matmul, rescale) tracks per-column log-offsets Z.
 - The t-dependency chain is latency-bound, so the sequence is processed
   meet-in-the-middle: an independent backward DP (beta recurrence, also
   linear) runs concurrently from t=L-1 down, and the two chains meet at
   t*=511 with a per-column dot product: score = log<alpha, beta> + Zf +
   Zb + const.  This halves the sequential chain length.
 - Variable sequence lengths are handled entirely on the host with no
   data-dependent device schedule: every sequence is shifted right so it
   ENDS at t=L-1, and the front is padded with features that hold the DP
   state exactly on the Perron eigenvector of E^T (pad value c - log
   lambda), with one boundary step handing off to the real chain.  The
   Perron structure makes the padding contribute exactly identity to the
   final score (absorbed into a per-column host constant).
 - The gold (numerator) score is a cheap O(B*L) gather computed on host.
"""

import os
import sys

import numpy as np

for _p in ("/opt/trn_rl_repo",):
    if _p not in sys.path and os.path.isdir(_p):
        sys.path.insert(0, _p)

B, L, T = 256, 1024, 128
NCORES = 8
BL = B // NCORES  # 32 sequences per core
TSTAR = 511  # forward covers t=0..511, backward t=1023..512
NORM_EVERY = int(os.environ.get("CRF_NORM_EVERY", "64"))
CHUNK = 32  # time steps per DMA/exp chunk
C_SHIFT = float(np.log(T) + 1.0)  # uniform per-step downshift folded into exp

# matmul operand dtype: "f32" (safe) or "bf16" (fast PE)
MM_DTYPE = os.environ.get("CRF_MM_DTYPE", "f32")

_PROGRAM_CACHE: dict = {}


def _build_program():
    """Build the (single, SPMD) Bass program shared by all 8 cores."""
    import concourse.bacc as bacc
    import concourse.bass as bass
    import concourse.mybir as mybir
    from concourse import tile

    f32 = mybir.dt.float32
    DT = mybir.dt.bfloat16 if MM_DTYPE == "bf16" else f32
    AF = mybir.ActivationFunctionType

    nc = bacc.Bacc("TRN2", target_bir_lowering=False, debug=False)

    fhat_d = nc.dram_tensor("fhat", (T, L * BL), f32, kind="ExternalInput")
    ef_d = nc.dram_tensor("ef", (T, T), DT, kind="ExternalInput")  # E (fwd lhsT)
    eb_d = nc.dram_tensor("eb", (T, T), DT, kind="ExternalInput")  # E^T (bwd lhsT)
    endc_d = nc.dram_tensor("endc", (1, BL), f32, kind="ExternalInput")
    out_d = nc.dram_tensor("scores", (1, BL), f32, kind="ExternalOutput")

    with tile.TileContext(nc) as tc:
        with (
            tc.tile_pool(name="const", bufs=1) as cp,
            tc.tile_pool(name="stage", bufs=int(os.environ.get("CRF_STAGE_BUFS", "3"))) as stp,
            tc.tile_pool(name="wp", bufs=int(os.environ.get("CRF_W_BUFS", "3"))) as wp,
            tc.tile_pool(name="vp", bufs=int(os.environ.get("CRF_W_BUFS", "3"))) as vp,
            tc.tile_pool(name="small", bufs=4) as sp,
            tc.tile_pool(name="puf", bufs=int(os.environ.get("CRF_PSUM_BUFS", "2")), space=bass.MemorySpace.PSUM) as puf,
            tc.tile_pool(name="pub", bufs=int(os.environ.get("CRF_PSUM_BUFS", "2")), space=bass.MemorySpace.PSUM) as pub,
            tc.tile_pool(name="ps", bufs=int(os.environ.get("CRF_PS_BUFS","2")), space=bass.MemorySpace.PSUM) as ps,
            tc.tile_pool(name="pr", bufs=int(os.environ.get("CRF_PS_BUFS","2")), space=bass.MemorySpace.PSUM) as pr,
        ):
            # --- constants / persistent state ---
            g = cp.tile((T, L * BL), DT, name="g")  # exp(Fhat - c) for all t
            ef_t = cp.tile((T, T), DT, name="ef_t")
            eb_t = cp.tile((T, T), DT, name="eb_t")
            endc_t = cp.tile((1, BL), f32, name="endc_t")
            ones_col = cp.tile((T, 1), DT, name="ones_col")
            ones_row = cp.tile((1, T), f32, name="ones_row")
            negc = cp.tile((T, 1), f32, name="negc")
            zero1 = cp.tile((1, 1), f32, name="zero1")
            zf = cp.tile((1, BL), f32, name="zf")
            zb = cp.tile((1, BL), f32, name="zb")

            nc.sync.dma_start(ef_t[:], ef_d[:])
            nc.sync.dma_start(eb_t[:], eb_d[:])
            nc.sync.dma_start(endc_t[:], endc_d[:])
            nc.vector.memset(ones_col[:], 1.0)
            nc.vector.memset(ones_row[:], 1.0)
            nc.vector.memset(negc[:], -C_SHIFT)
            nc.vector.memset(zero1[:], 0.0)
            nc.vector.memset(zf[:], 0.0)
            nc.vector.memset(zb[:], 0.0)

            # --- stream Fhat in chunks, exp into g (ACT) ---
            # fwd consumes chunks 0,1,2,... ; bwd consumes 31,30,...
            n_chunks = L // CHUNK
            order = []
            lo, hi = 0, n_chunks - 1
            while lo <= hi:
                order.append(lo)
                if hi != lo:
                    order.append(hi)
                lo += 1
                hi -= 1
            for k in order:
                c0, c1 = k * CHUNK * BL, (k + 1) * CHUNK * BL
                st = stp.tile((T, CHUNK * BL), f32, tag="st", name="st")
                nc.sync.dma_start(st[:], fhat_d[:, c0:c1])
                nc.scalar.activation(g[:, c0:c1], st[:], AF.Exp, bias=negc[:])

            def norm(x_ap, z_tile):
                """Rescale columns of x by 1/colsum; accumulate log colsum in z."""
                s = ps.tile((1, BL), f32, tag="s", name="s")
                nc.tensor.matmul(s[:], ones_col[:], x_ap, start=True, stop=True)
                r = sp.tile((1, BL), f32, tag="r", name="r")
                nc.vector.reciprocal(r[:], s[:])
                rb = pr.tile((T, BL), f32, tag="rb", name="rb")
                nc.tensor.matmul(rb[:], ones_row[:], r[:], start=True, stop=True)
                nc.vector.tensor_mul(x_ap, x_ap, rb[:])
                ls = sp.tile((1, BL), f32, tag="ls", name="ls")
                nc.scalar.activation(ls[:], s[:], AF.Ln, bias=zero1[:])
                nc.vector.tensor_add(z_tile[:], z_tile[:], ls[:])

            # --- the two DP chains, interleaved ---
            w_prev = g[:, 0:BL]  # alpha_0 = exp(Fhat_0 - c) (host pre-biased)
            v_prev = g[:, (L - 1) * BL : L * BL]  # v_{L-1} = g_{L-1} * 1
            for tk in range(1, TSTAR + 1):
                tf = tk
                tb = L - 1 - tk  # 1022 .. 512
                uf = puf.tile((T, BL), f32, tag="uf", name="uf")
                nc.tensor.matmul(uf[:], ef_t[:], w_prev, start=True, stop=True)
                w = wp.tile((T, BL), DT, tag="w", name="w")
                nc.vector.tensor_mul(w[:], uf[:], g[:, tf * BL : (tf + 1) * BL])
                w_prev = w[:]

                ub = pub.tile((T, BL), f32, tag="ub", name="ub")
                nc.tensor.matmul(ub[:], eb_t[:], v_prev, start=True, stop=True)
                v = vp.tile((T, BL), DT, tag="v", name="v")
                nc.vector.tensor_mul(v[:], ub[:], g[:, tb * BL : (tb + 1) * BL])
                v_prev = v[:]

                if tk % NORM_EVERY == 0:
                    norm(w_prev, zf)
                    norm(v_prev, zb)

            # --- meet: score = log<w_511, E v_512> + Zf + Zb + endc ---
            um = pub.tile((T, BL), f32, tag="ub", name="um")
            nc.tensor.matmul(um[:], eb_t[:], v_prev, start=True, stop=True)
            d = wp.tile((T, BL), DT, tag="w", name="d")
            nc.vector.tensor_mul(d[:], um[:], w_prev)
            dots = ps.tile((1, BL), f32, tag="s", name="dots")
            nc.tensor.matmul(dots[:], ones_col[:], d[:], start=True, stop=True)
            lnd = sp.tile((1, BL), f32, tag="ls", name="lnd")
            nc.scalar.activation(lnd[:], dots[:], AF.Ln, bias=zero1[:])
            sc = sp.tile((1, BL), f32, tag="sc", name="sc")
            nc.vector.tensor_add(sc[:], lnd[:], zf[:])
            nc.vector.tensor_add(sc[:], sc[:], zb[:])
            nc.vector.tensor_add(sc[:], sc[:], endc_t[:])
            nc.sync.dma_start(out_d[:], sc[:])

    nc.compile()
    return nc


def _get_program():
    key = MM_DTYPE
    if key not in _PROGRAM_CACHE:
        _PROGRAM_CACHE[key] = _build_program()
    return _PROGRAM_CACHE[key]


def _host_prep(transitions, feats, mask):
    """Shift sequences to end at t=L-1; Perron-pad fronts; per-b constants."""
    tr64 = transitions.astype(np.float64)
    E = np.exp(tr64)
    # Perron eigenpair of E^T (power iteration in f64: positive matrix)
    rho = np.ones(T, np.float64) / T
    for _ in range(200):
        rho = E.T @ rho
        rho /= rho.sum()
    lam = float((E.T @ rho).sum())  # since rho sums to 1
    logrho = np.log(rho)
    loglam = np.log(lam)
    c = C_SHIFT

    lengths = mask.sum(axis=1).astype(np.int64)
    start = L - lengths

    f64 = feats.astype(np.float64)
    Fhat = np.empty((B, L, T), np.float64)
    # pad region default; overwritten below where real/boundary
    for b in range(B):
        s = int(start[b])
        if s == 0:
            Fhat[b, 0] = f64[b, 0]
            Fhat[b, 1:] = f64[b, 1:]
        else:
            Fhat[b, 0] = logrho
            Fhat[b, 1:s] = c - loglam
            Fhat[b, s] = f64[b, 0] + c - loglam - logrho
            Fhat[b, s + 1 :] = f64[b, 1 : int(lengths[b])]
    # device applies exp(x - c) uniformly incl. t=0 -> bake +c into t=0
    Fhat[:, 0, :] += c

    endc = (lengths - 1).astype(np.float64) * c
    return Fhat.astype(np.float32), endc.astype(np.float32), E


def _host_gold(transitions, feats, mask, tags):
    f64 = feats.astype(np.float64)
    tr64 = transitions.astype(np.float64)
    tags_i = tags.astype(np.int64)
    unary = np.take_along_axis(f64, tags_i[:, :, None], axis=2)[..., 0]
    unary_sum = np.where(mask, unary, 0.0).sum()
    binary = tr64[tags_i[:, :-1], tags_i[:, 1:]]
    binary_sum = np.where(mask[:, 1:], binary, 0.0).sum()
    return unary_sum + binary_sum


def kernel(transitions, feats, mask, tags):
    import ml_dtypes
    from concourse import bass_utils

    transitions = np.asarray(transitions, np.float32)
    feats = np.asarray(feats, np.float32)
    mask = np.asarray(mask).astype(bool)

    Fhat, endc, E = _host_prep(transitions, feats, mask)

    np_dt = ml_dtypes.bfloat16 if MM_DTYPE == "bf16" else np.float32
    E_f = E.astype(np_dt)  # lhsT fwd: out = E^T w
    E_b = E.T.astype(np_dt).copy()  # lhsT bwd: out = E v

    in_maps = []
    for k in range(NCORES):
        sl = slice(k * BL, (k + 1) * BL)
        # layout Fhat_core[j, t*BL + b] = Fhat[b0+b, t, j]
        fh = np.ascontiguousarray(
            Fhat[sl].transpose(2, 1, 0).reshape(T, L * BL)
        )
        in_maps.append(
            {
                "fhat": fh,
                "ef": E_f,
                "eb": E_b,
                "endc": endc[sl].reshape(1, BL),
            }
        )

    nc = _get_program()
    res = bass_utils.run_bass_kernel_spmd(nc, in_maps, core_ids=list(range(NCORES)))
    fwd_total = 0.0
    for k in range(NCORES):
        fwd_total += res.results[k]["scores"].astype(np.float64).sum()

    gold = _host_gold(transitions, feats, mask, tags)
    loss = fwd_total - gold
    return np.float32(loss)
